# revision 9
# baseline (speedup 1.0000x reference)
"""GPTQ 4-bit quantized linear (CaiQuantLinear) on 8 TRN2 NeuronCores.

Computes out = x @ dequant(qweight, scales, qzeros) + bias where
  x: (4, 2048, 4096) fp16, qweight: (512, 4096) int32 (8x 4-bit per word,
  packed along input features), scales: (32, 4096) fp16, qzeros: (32, 512)
  int32 (packed along output features), bias: (4096,) fp16.
  Groups are contiguous blocks of 128 input features (g_idx = arange//128).

Sharding: tensor-parallel column split over output features. Each of the 8
cores gets 512 output columns (its slice of qweight/scales/qzeros/bias) and
the full x (replicated). No collectives; the host concatenates the 8 column
slices.

Host-side layout prep (pure data movement, no arithmetic): x is transposed
to [in, seq] so the device loads k-major lhsT tiles with large contiguous
packets instead of element-scatter transpose DMA; qweight rows are
replicated 8x so partition p of a k-tile holds the packed word for feature
p, unpacked in place with a per-partition shift.

Per-core kernel:
  1. Dequant (no PE involvement): unpack qzeros to a z+1 table [32, 512]
     int32, round-trip through DRAM so each group row can be broadcast to
     128 partitions. Per k-tile: natural load of the 8x-expanded qweight
     words, per-partition-shift unpack (vector), int32 subtract of the
     broadcast z1 row casting to fp16 (vector), fp16 multiply by the
     broadcast scale row (gpsimd) directly into the resident w_all
     [128, 32 k-tiles, 512 out] fp16.
  2. Matmul: 16 chunks of 512 seq positions; per chunk 4 PSUM banks
     accumulate over the 32 k-tiles (lhsT = xT 128x128 block, rhs = w_all
     k-slice 512 wide). Chunks ping-pong two 4-bank PSUM sets so the next
     chunk's matmuls overlap the previous drain. The first two chunks are
     interleaved k-wave by k-wave with the dequant so the PE starts
     immediately. Bias is added fp32 on the PSUM drain (vector); stores go
     via gpsimd SWDGE. x tiles stream on both HWDGE rings (sync/scalar)
     three chunks ahead.
"""

import sys

if "/opt/trn_rl_repo" not in sys.path:
    sys.path.insert(0, "/opt/trn_rl_repo")

import numpy as np

B, S, IN, OUT = 4, 2048, 4096, 4096
SEQ = B * S                      # 8192
NCORES = 8
OUT_S = OUT // NCORES            # 512 output columns per core
PACK = 8                         # int32 packs 8 nibbles
GSIZE = 128                      # group size == k-tile size
CHUNK = 512                      # seq positions per PSUM chunk

_CACHE = {}


def _build(seq, in_f, out_s, chunk):
    """Build + compile the per-core Bass program. All cores run the same
    NEFF on their own input slices (SPMD, no collectives)."""
    import concourse.bass as bass  # noqa: F401
    import concourse.mybir as mybir
    import concourse.tile as tile
    from concourse import bacc

    dt = mybir.dt
    op = mybir.AluOpType
    P = 128
    KT = in_f // P                # k-tiles (== groups) = 32
    CH = seq // chunk             # chunks = 16
    ST = chunk // P               # psum tiles per chunk = 4

    nc = bacc.Bacc("TRN2", target_bir_lowering=False, debug=False,
                   num_devices=NCORES)

    xT_d = nc.dram_tensor("xT", (in_f, seq), dt.float16, kind="ExternalInput")
    qb_d = nc.dram_tensor("qbig", (in_f, out_s), dt.int16,
                          kind="ExternalInput")
    sc_d = nc.dram_tensor("scales", (KT, out_s), dt.float16,
                          kind="ExternalInput")
    qz_d = nc.dram_tensor("qzeros", (KT, out_s // PACK), dt.int32,
                          kind="ExternalInput")
    b_d = nc.dram_tensor("bias", (1, out_s), dt.float16, kind="ExternalInput")
    sh_d = nc.dram_tensor("shifts", (P, 1), dt.int16, kind="ExternalInput")
    out_d = nc.dram_tensor("out", (seq, out_s), dt.float16,
                           kind="ExternalOutput")

    xT = xT_d.ap()
    qb = qb_d.ap()
    scales = sc_d.ap()
    qzeros = qz_d.ap()
    bias = b_d.ap()
    out = out_d.ap()

    with tile.TileContext(nc) as tc:
        with (
            tc.tile_pool(name="const", bufs=1) as const_pool,
            tc.tile_pool(name="w", bufs=1) as w_pool,
            tc.tile_pool(name="qk", bufs=3) as qk_pool,
            tc.tile_pool(name="zb", bufs=3) as zb_pool,
            tc.tile_pool(name="sb", bufs=3) as sb_pool,
            tc.tile_pool(name="wi", bufs=3) as wi_pool,
            tc.tile_pool(name="d16", bufs=3) as d_pool,
            tc.tile_pool(name="xt", bufs=96) as xt_pool,
            tc.tile_pool(name="ot", bufs=6) as out_pool,
            tc.tile_pool(name="ps", bufs=8, space="PSUM") as psum_pool,
            tc.tile_pool(name="dram", bufs=1, space="DRAM") as dram_pool,
        ):
            # ---- z+1 table first: it gates the whole dequant chain.
            # Unpack qzeros, +1, cast to int16, round-trip to DRAM so the
            # per-group rows can be partition-broadcast ----
            qz_sb = const_pool.tile([KT, out_s // PACK], dt.int32)
            nc.sync.dma_start(qz_sb, qzeros)
            shifts = const_pool.tile([P, 1], dt.int16)
            nc.sync.dma_start(shifts, sh_d.ap())
            z_i = const_pool.tile([KT, out_s], dt.int32)
            z_iv = z_i.rearrange("g (c s) -> g c s", s=PACK)
            for s in range(PACK):
                nc.vector.tensor_scalar(
                    out=z_iv[:, :, s], in0=qz_sb, scalar1=4 * s, scalar2=0xF,
                    op0=op.logical_shift_right, op1=op.bitwise_and)
            ones = const_pool.tile([KT, out_s], dt.int32)
            nc.vector.memset(ones, 1)
            z1_i = const_pool.tile([KT, out_s], dt.int32)
            nc.vector.tensor_add(z1_i, z_i, ones)
            z1_16 = const_pool.tile([KT, out_s], dt.int16)
            nc.vector.tensor_copy(z1_16, z1_i)
            z1_d = dram_pool.tile([KT, out_s], dt.int16)
            nc.gpsimd.dma_start(z1_d, z1_16)

            # bias tiles are only needed at the first drain (~80us in);
            # emitted after the z-table so they never block it
            bias16 = const_pool.tile([P, out_s], dt.float16)
            bias32 = const_pool.tile([P, out_s], dt.float32)

            # fp16 weights stay resident: w_all[:, k, :] is k-tile k
            w_all = w_pool.tile([P, KT, out_s], dt.float16)

            # ---- x streaming / matmul helpers ----
            xts = {}

            def load_chunk(c):
                lst = []
                for k in range(KT):
                    t = xt_pool.tile([P, chunk], dt.float16, tag="xt",
                                     name=f"xt_{c}_{k}")
                    eng = nc.sync if (k % 2 == 0) else nc.scalar
                    eng.dma_start(
                        t, xT[k * P:(k + 1) * P, c * chunk:(c + 1) * chunk])
                    lst.append(t)
                xts[c] = lst

            pss = {}

            def alloc_ps(c):
                pss[c] = [psum_pool.tile([P, out_s], dt.float32, tag="acc",
                                         name=f"ps_{c}_{st}")
                          for st in range(ST)]

            def mm(c, k):
                for st in range(ST):
                    nc.tensor.matmul(
                        pss[c][st],
                        lhsT=xts[c][k][:, st * P:(st + 1) * P],
                        rhs=w_all[:, k, :],
                        start=(k == 0), stop=(k == KT - 1))

            def drain(c):
                # the last chunk's stores go on the (by then idle) HWDGE
                # rings so the tail doesn't pay the SWDGE flush
                last = c == CH - 1
                for st in range(ST):
                    o16 = out_pool.tile([P, out_s], dt.float16, tag="o16",
                                        name=f"o16_{c}_{st}")
                    nc.vector.tensor_add(o16, pss[c][st], bias32)
                    r0 = c * chunk + st * P
                    if last:
                        eng = nc.sync if st % 2 == 0 else nc.scalar
                    else:
                        eng = nc.gpsimd
                    eng.dma_start(out[r0:r0 + P, :], o16)
                del pss[c]
                del xts[c]

            # ---- dequant loop, interleaved with chunks 0 and 1 ----
            # Per k-wave the dequant DMAs go at the ring head (in-order
            # rings: x tiles queued ahead would starve the dequant chain),
            # the k's own x tiles ride just behind, and the byte load is
            # mirrored by k parity so both HWDGE rings carry equal traffic.
            xts[0] = []
            xts[1] = []
            alloc_ps(0)
            alloc_ps(1)
            for k in range(KT):
                ea = nc.sync if k % 2 == 0 else nc.scalar
                eb = nc.scalar if k % 2 == 0 else nc.sync
                qk = qk_pool.tile([P, out_s], dt.int16, tag="qk")
                ea.dma_start(qk, qb[k * P:(k + 1) * P, :])
                z1bc = zb_pool.tile([P, out_s], dt.int16, tag="zb")
                eb.dma_start(
                    z1bc, z1_d[k:k + 1, :].to_broadcast((P, out_s)))
                sbc = sb_pool.tile([P, out_s], dt.float16, tag="sb")
                eb.dma_start(
                    sbc, scales[k:k + 1, :].to_broadcast((P, out_s)))
                for c in (0, 1):
                    t = xt_pool.tile([P, chunk], dt.float16, tag="xt",
                                     name=f"xt_{c}_{k}")
                    (ea if c == 0 else eb).dma_start(
                        t, xT[k * P:(k + 1) * P, c * chunk:(c + 1) * chunk])
                    xts[c].append(t)
                wi16 = wi_pool.tile([P, out_s], dt.int16, tag="wi")
                nc.vector.tensor_scalar(
                    out=wi16, in0=qk, scalar1=shifts, scalar2=0xF,
                    op0=op.logical_shift_right, op1=op.bitwise_and)
                d16 = d_pool.tile([P, out_s], dt.float16, tag="d16")
                nc.vector.tensor_tensor(
                    out=d16, in0=wi16, in1=z1bc, op=op.subtract)
                # alternate the dequant multiply between the two DVE-class
                # engines; gpsimd TT is ~2x slower so it takes every other k
                meng = nc.vector if k % 2 == 0 else nc.gpsimd
                meng.tensor_mul(w_all[:, k, :], d16, sbc)
                mm(0, k)
                mm(1, k)
                if k == 2:
                    # bias prep rides here: off the critical dequant path,
                    # ready long before the first drain
                    nc.scalar.dma_start(bias16, bias.to_broadcast((P, out_s)))
                    nc.vector.tensor_copy(bias32, bias16)
            drain(0)
            drain(1)

            # ---- steady-state chunks ----
            for c in range(2, CH):
                load_chunk(c)
                alloc_ps(c)
                for k in range(KT):
                    mm(c, k)
                drain(c)

    nc.compile()
    return nc


def _get_program(seq, in_f, out_s, chunk):
    key = (seq, in_f, out_s, chunk)
    if key not in _CACHE:
        _CACHE[key] = _build(seq, in_f, out_s, chunk)
    return _CACHE[key]


def _make_in_maps(x, qweight, scales, qzeros, bias):
    """Host-side sharding + layout prep shared by kernel() and test.py.

    Layout only, no arithmetic: x transposed; qweight viewed as int16
    halves and gathered so row k holds the half-word containing feature
    k's nibble (little-endian: half 0 = bits 0-15 = nibbles 0-3)."""
    x2 = np.asarray(x).reshape(SEQ, IN)
    xT = np.ascontiguousarray(x2.T)                      # [IN, SEQ]
    qweight = np.asarray(qweight)
    scales = np.asarray(scales)
    qzeros = np.asarray(qzeros)
    bias = np.asarray(bias)
    sh = ((np.arange(128) % 4) * 4).astype(np.int16).reshape(128, 1)
    kk = np.arange(IN)

    zcols = OUT_S // PACK
    in_maps = []
    for c in range(NCORES):
        o0 = c * OUT_S
        qv = np.ascontiguousarray(qweight[:, o0:o0 + OUT_S]).view(
            np.int16).reshape(IN // PACK, OUT_S, 2)
        qb16 = np.ascontiguousarray(qv[kk // PACK, :, (kk % PACK) // 4])
        in_maps.append({
            "xT": xT,
            "qbig": qb16,                                # [IN, OUT_S] int16
            "scales": np.ascontiguousarray(scales[:, o0:o0 + OUT_S]),
            "qzeros": np.ascontiguousarray(
                qzeros[:, c * zcols:(c + 1) * zcols]),
            "bias": np.ascontiguousarray(
                bias[o0:o0 + OUT_S].reshape(1, OUT_S)),
            "shifts": sh,
        })
    return in_maps


def kernel(x, qweight, scales, qzeros, g_idx=None, bias=None, **_unused):
    """Full-input entry point: shards over 8 cores, runs on HW, gathers."""
    from concourse.bass_utils import run_bass_kernel_spmd

    nc = _get_program(SEQ, IN, OUT_S, CHUNK)
    in_maps = _make_in_maps(x, qweight, scales, qzeros, bias)

    res = run_bass_kernel_spmd(nc, in_maps, core_ids=list(range(NCORES)))
    full = np.concatenate([res.results[c]["out"] for c in range(NCORES)],
                          axis=1)
    return full.reshape(B, S, OUT).astype(np.float16)
